# revision 1
# baseline (speedup 1.0000x reference)
"""Trainium2 Bass kernel for multi-head attention (B=2, T=2048, C=1024, H=16, DH=64).

Sharding: tensor-parallel over heads. Each of the 8 cores computes 2 heads:
q/k/v projections for its heads, attention, and a partial output projection
(its 128-column slice of the concat-head dim against its 128-row slice of Wp).
The host sums the 8 partial outputs and adds the bias.

Device layout choices:
  - x is pre-transposed on the host to xT (C, B*T) so the projection matmuls
    can produce qT/kT/vT in (head*dh, tokens) layout directly.
  - scores are computed transposed (keys on partitions, queries on free dim)
    so the key-validity mask is a per-partition bias fused into the exp
    activation: expT = Exp(scoresT * 1/8 + mask_bias).
  - the softmax denominator comes for free from an appended ones-column on V
    (v_aug = [v | 1]); normalization is a reciprocal + gpsimd
    partition-broadcast + elementwise multiply.
  - all matmuls run as float32r (fp32 data, FP22 multiply) for full PE rate.
  - query/key ranges are specialized on the runtime `lengths` values (the
    module is rebuilt per distinct tile-count signature; invalid token rows
    are never computed and are zeroed host-side).
"""

from contextlib import ExitStack

import numpy as np

B, T, C, H, DH = 2, 2048, 1024, 16, 64
NCORES = 8
HP = H // NCORES          # heads per core
M = HP * DH               # 128 = packed head dim per core
P = 128                   # partitions / contraction chunk
QT = 512                  # query/token tile (free dim per matmul)
NEG = -30000.0            # additive mask value (exp(NEG + anything small) == 0)


def _build(lens, t=T, c=C):
    """Build the per-core Bass module for batch lengths `lens` (tuple of B ints)."""
    import concourse.mybir as mybir
    import concourse.tile as tile
    from concourse import bacc
    from concourse.masks import make_identity

    f32 = mybir.dt.float32
    f32r = mybir.dt.float32r
    AF = mybir.ActivationFunctionType

    qt = min(QT, t)
    kc_n = c // P                       # contraction chunks for projections
    nkv = [(l + P - 1) // P for l in lens]        # valid key chunks
    nq = [(l + qt - 1) // qt for l in lens]       # valid query tiles
    nob = [(l + 127) // 128 for l in lens]        # valid output token blocks
    nkv_max = max(nkv)

    nc = bacc.Bacc("TRN2", target_bir_lowering=False, debug=False,
                   num_devices=NCORES)

    xt_d = nc.dram_tensor("xt", [c, B * t], f32, kind="ExternalInput").ap()
    wq_d = nc.dram_tensor("wq", [c, M], f32, kind="ExternalInput").ap()
    wk_d = nc.dram_tensor("wk", [c, M], f32, kind="ExternalInput").ap()
    wv_d = nc.dram_tensor("wv", [c, M], f32, kind="ExternalInput").ap()
    wp_d = nc.dram_tensor("wp", [M, c], f32, kind="ExternalInput").ap()
    km_d = nc.dram_tensor("km", [P, B], f32, kind="ExternalInput").ap()
    out_d = nc.dram_tensor("out", [B * t, c], f32, kind="ExternalOutput").ap()

    xt_r = xt_d.rearrange("(kc p) t -> p kc t", p=P)

    with tile.TileContext(nc) as tc, ExitStack() as ctx:
        const = ctx.enter_context(tc.tile_pool(name="const", bufs=1))
        persist = ctx.enter_context(tc.tile_pool(name="persist", bufs=1))

        ident = const.tile([P, P], f32)
        make_identity(nc, ident[:])
        kmask = const.tile([P, B], f32)
        wp_sb = const.tile([P, c], f32r)

        qT = persist.tile([P, B * t], f32r, tag="qT")
        kT = persist.tile([P, B * t], f32r, tag="kT")
        vT = persist.tile([P, B * t], f32, tag="vT")
        # v_aug[:, b, k, 0:64+1]: per key chunk, [v_head | ones]; one per head
        vaug = [persist.tile([P, B, nkv_max, DH + 1], f32r, tag=f"vaug{h}",
                             name=f"vaug{h}")
                for h in range(HP)]

        # Per-batch pipeline: projections(b) -> v_aug(b) -> attention(b).
        # Batch b+1's projections overlap batch b's attention (ACT-bound).
        # PSUM budget (8 banks): work1b 2 + psc 4 + attn shared 2.
        qn = min(QT, c)                  # output-projection free-dim tile
        with tc.tile_pool(name="wpool", bufs=1) as wpool, \
             tc.tile_pool(name="xpool", bufs=3 if nkv_max <= 14 else 2) as xpool, \
             tc.tile_pool(name="exps", bufs=min(nkv_max + 2, 18)) as expp, \
             tc.tile_pool(name="stage", bufs=2) as stage, \
             tc.tile_pool(name="aop", bufs=2) as aop, \
             tc.tile_pool(name="recp", bufs=2) as recp, \
             tc.tile_pool(name="ppj", bufs=2, space="PSUM") as ppj, \
             tc.tile_pool(name="psc", bufs=2, space="PSUM") as psc, \
             tc.tile_pool(name="pat", bufs=2, space="PSUM") as pat:
            w_sb = []
            w_dr = []
            for name, wd in (("wq", wq_d), ("wk", wk_d), ("wv", wv_d)):
                w = wpool.tile([P, kc_n, M], f32r, tag=name, name=name + "s")
                w_sb.append(w)
                w_dr.append(wd.rearrange("(kc p) m -> p kc m", p=P).bitcast(f32r))
            for h in range(HP):
                # ones everywhere; v copies then overwrite cols 0:DH per chunk,
                # leaving col DH as the denominator ones-column
                nc.vector.memset(vaug[h][:].bitcast(f32), 1.0)

            # Warm-up during the initial DMA wait: dependency-free matmuls on
            # the identity tile release the PE's HAM clock gate (~3.4us of
            # activity), a dummy Exp preloads the ACT table set (~2.7us
            # otherwise paid at the first real exp), and a dummy broadcast
            # warms the gpsimd custom-op path
            warm = ppj.tile([P, qt], f32, tag="w1b", name="warm")
            for i in range(17):
                nc.tensor.matmul(warm[:, 0:P], ident[:], ident[:],
                                 start=(i == 0), stop=(i == 16))
            dummy = const.tile([P, P], f32, name="dummy")
            nc.scalar.activation(dummy[:], ident[:], AF.Exp)
            nc.gpsimd.partition_broadcast(dummy[0:DH, :], ident[0:1, :])

            # tile width for token/query tile i of batch b: cover the valid
            # range rounded up to full 128-key chunks (so kT/vT have no
            # garbage inside chunk coverage); >=256 keeps float32r full-rate
            def tiw(b, i):
                cov = min(nkv[b] * P - i * qt, qt)
                return max(256, min(qt, (cov + 63) // 64 * 64))

            def emit_proj_tile(b, n):
                    tok0 = b * t + n * qt
                    tw = tiw(b, n)
                    xtile = xpool.tile([P, kc_n, qt], f32r, tag="x", name="xtile")
                    if b == 0 and n == 0:
                        # first wq, then the first x tile in two pieces, then
                        # the rest — orders HWDGE work so matmuls start early
                        nc.sync.dma_start(w_sb[0][:], w_dr[0][:])
                        nc.sync.dma_start(
                            xtile[:, 0:2, 0:tw],
                            xt_r[:, 0:2, tok0:tok0 + tw].bitcast(f32r))
                        nc.sync.dma_start(w_sb[1][:], w_dr[1][:])
                        nc.sync.dma_start(w_sb[2][:], w_dr[2][:])
                        mid = 2 + (kc_n - 2) // 2
                        nc.sync.dma_start(
                            xtile[:, 2:mid, 0:tw],
                            xt_r[:, 2:mid, tok0:tok0 + tw].bitcast(f32r))
                        nc.sync.dma_start(
                            xtile[:, mid:kc_n, 0:tw],
                            xt_r[:, mid:kc_n, tok0:tok0 + tw].bitcast(f32r))
                        nc.sync.dma_start(kmask[:], km_d[:])
                        nc.sync.dma_start(wp_sb[:], wp_d[:].bitcast(f32r))
                    elif b == 0 and n == 1:
                        half = kc_n // 2
                        nc.sync.dma_start(
                            xtile[:, 0:half, 0:tw],
                            xt_r[:, 0:half, tok0:tok0 + tw].bitcast(f32r))
                        nc.sync.dma_start(
                            xtile[:, half:kc_n, 0:tw],
                            xt_r[:, half:kc_n, tok0:tok0 + tw].bitcast(f32r))
                    else:
                        nc.sync.dma_start(
                            xtile[:, :, 0:tw],
                            xt_r[:, :, tok0:tok0 + tw].bitcast(f32r))
                    for w, dstT in zip(w_sb, (qT, kT, vT)):
                        ps = ppj.tile([P, qt], f32, tag="w1b", name="ps")
                        for k in range(kc_n):
                            nc.tensor.matmul(
                                ps[:, 0:tw], w[:, k, :], xtile[:, k, 0:tw],
                                start=(k == 0), stop=(k == kc_n - 1))
                        nc.vector.tensor_copy(dstT[:, tok0:tok0 + tw],
                                              ps[:, 0:tw])

            def emit_proj(b):
                for n in range(nq[b]):
                    emit_proj_tile(b, n)

            def emit_vaug_chunks(b, k0, k1):
                # one 128x128 transpose covers both heads' v for the chunk
                for k in range(k0, k1):
                    key0 = b * t + k * P
                    pt = ppj.tile([P, qt], f32, tag="w1b", name="pt")
                    nc.tensor.transpose(
                        pt[:, 0:P], vT[:, key0:key0 + P], ident[:])
                    for h in range(HP):
                        nc.vector.tensor_copy(vaug[h][:, b, k, 0:DH],
                                              pt[:, h * DH:(h + 1) * DH])

            def emit_vaug(b):
                emit_vaug_chunks(b, 0, nkv[b])

            def hoff(qw):
                # head-1 offset must stay bank-aligned: concurrent row-tiled
                # matmuls to the same psum bank fault on hardware
                return qt

            def emit_scores_chunks(b, q, k0, k1, etiles):
                partial = lens[b] % P != 0
                q0 = b * t + q * qt
                qw = tiw(b, q)
                off1 = hoff(qw)
                # scoresT per key chunk (both heads row-tiled), then exp with
                # fused 1/sqrt(dh) scale + key-mask bias; one ACT op spans
                # both heads (any [qw:off1] gap is never read downstream)
                for k in range(k0, k1):
                    key0 = b * t + k * P
                    ps = psc.tile([P, 2 * qt], f32, tag="sc", name="psck")
                    for h in range(HP):
                        nc.tensor.matmul(
                            ps[:, h * off1:h * off1 + qw],
                            kT[h * DH:(h + 1) * DH, key0:key0 + P],
                            qT[h * DH:(h + 1) * DH, q0:q0 + qw],
                            start=True, stop=True,
                            tile_position=(h * DH, 0))
                    et = expp.tile([P, 2 * qt], f32r, tag="et", name="et")
                    bias = kmask[:, b:b + 1] if (partial and k == nkv[b] - 1) \
                        else 0.0
                    # strided AP covers exactly both heads' written regions
                    # (no dead-gap elements when qw < qt)
                    src = ps[:].rearrange("p (g w) -> p g w", g=2)[:, :, 0:qw]
                    dst = et[:].rearrange("p (g w) -> p g w", g=2)[:, :, 0:qw]
                    nc.scalar.activation(dst, src, AF.Exp,
                                         bias=bias, scale=0.125)
                    etiles.append(et)

            def emit_scores(b, q):
                etiles = []
                emit_scores_chunks(b, q, 0, nkv[b], etiles)
                return etiles

            def emit_av(b, q, etiles):
                q0 = b * t + q * qt
                qw = tiw(b, q)
                aot = aop.tile([P, qt], f32r, tag="ao", name="aot")
                off1 = hoff(qw)
                # attention @ v_aug (accumulate over key chunks), then
                # normalize by the ones-column denominator
                for h in range(HP):
                    pa = pat.tile([P, qt], f32, tag="at", name="pa")
                    for k in range(nkv[b]):
                        nc.tensor.matmul(
                            pa[0:DH + 1, 0:qw],
                            vaug[h][:, b, k, :],
                            etiles[k][:, h * off1:h * off1 + qw],
                            start=(k == 0), stop=(k == nkv[b] - 1))
                    rec = recp.tile([1, qt], f32, tag="rec", name="rec")
                    nc.vector.reciprocal(rec[:, 0:qw], pa[DH:DH + 1, 0:qw])
                    rbc = recp.tile([DH, qt], f32, tag="rbc", name="rbc")
                    nc.gpsimd.partition_broadcast(rbc[:, 0:qw], rec[:, 0:qw])
                    nc.vector.tensor_mul(aot[h * DH:(h + 1) * DH, 0:qw],
                                         pa[0:DH, 0:qw], rbc[:, 0:qw])
                return aot

            def emit_outproj(b, q, aot):
                final_bq = b == B - 1 and q == nq[B - 1] - 1
                # output projection for the token blocks of this query tile
                for j in range(qt // 128):
                    blk = q * (qt // 128) + j
                    if blk >= nob[b]:
                        break
                    tok0 = b * t + blk * 128
                    st = stage.tile([P, c], f32, tag="st", name="st")
                    for nn in range(c // qn):
                        po = ppj.tile([P, qn], f32, tag="w1b", name="po")
                        nc.tensor.matmul(
                            po[:],
                            aot[:, j * 128:(j + 1) * 128],
                            wp_sb[:, nn * qn:(nn + 1) * qn],
                            start=True, stop=True)
                        nc.any.tensor_copy(st[:, nn * qn:(nn + 1) * qn],
                                           po[:])
                    if final_bq and blk == nob[b] - 1 and c == 2 * qn:
                        # last block on the exit-critical path: pipeline the
                        # evacuation + DMA in halves
                        nc.sync.dma_start(out_d[tok0:tok0 + 128, 0:qn],
                                          st[:, 0:qn])
                        nc.sync.dma_start(out_d[tok0:tok0 + 128, qn:c],
                                          st[:, qn:c])
                    else:
                        nc.sync.dma_start(out_d[tok0:tok0 + 128, :], st[:])

            # emission order = scheduling priority: batch b+1's projections
            # are emitted inside batch b's first query tile (between scores
            # and attnV) so the PE fills ACT-paced stretches with
            # next-batch projection work
            # Scheduling is static per engine, so emission order decides what
            # the PE blocks on. Batch 0's first score chunks are interleaved
            # between its projection tiles (2 at a time, matching the score
            # psum double-buffer) so the DMA-gated projection phase keeps the
            # PE fed and the exp stream starts early; v_aug transposes fill
            # the remaining score-psum stalls. Each tile's output projection
            # is deferred until after the next tile's scores.
            etiles0 = []
            k_sc = 0
            for n in range(nq[0]):
                emit_proj_tile(0, n)
                hi = min(((n + 1) * qt) // P, nkv[0])
                take = min(k_sc + 2, hi)
                emit_scores_chunks(0, 0, k_sc, take, etiles0)
                k_sc = take
            k_va = 0
            while k_sc < nkv[0] or k_va < nkv[0]:
                take = min(k_sc + 2, nkv[0])
                emit_scores_chunks(0, 0, k_sc, take, etiles0)
                k_sc = take
                take = min(k_va + 3, nkv[0])
                emit_vaug_chunks(0, k_va, take)
                k_va = take

            pend = []
            # spread next-batch projection tiles between attention steps
            ptiles = list(range(nq[1])) if B > 1 else []
            for b in range(B):
                for q in range(nq[b]):
                    ets = etiles0 if (b == 0 and q == 0) else emit_scores(b, q)
                    if b == 0 and ptiles:
                        emit_proj_tile(1, ptiles.pop(0))
                    if pend:
                        emit_outproj(*pend.pop(0))
                    pend.append((b, q, emit_av(b, q, ets)))
                    if b == 0 and ptiles:
                        emit_proj_tile(1, ptiles.pop(0))
                    if b + 1 < B and q == nq[b] - 1:
                        for n in ptiles:
                            emit_proj_tile(1, n)
                        ptiles = []
                        emit_vaug(1)
            while pend:
                emit_outproj(*pend.pop(0))

    nc.compile()
    return nc


_module_cache = {}


def _get_module(lens):
    key = tuple((l + P - 1) // P for l in lens) + tuple(l % P == 0 for l in lens)
    if key not in _module_cache:
        _module_cache[key] = _build(lens)
    return _module_cache[key]


def kernel(x, lengths, Wq, Wk, Wv, Wp, bp):
    from concourse.bass_utils import run_bass_kernel_spmd

    x = np.asarray(x, dtype=np.float32)
    lens = tuple(int(np.clip(int(v), 1, T)) for v in np.asarray(lengths).reshape(-1))
    Wq = np.asarray(Wq, dtype=np.float32)
    Wk = np.asarray(Wk, dtype=np.float32)
    Wv = np.asarray(Wv, dtype=np.float32)
    Wp = np.asarray(Wp, dtype=np.float32)
    bp = np.asarray(bp, dtype=np.float32)

    nc = _get_module(lens)

    xt = np.ascontiguousarray(x.reshape(B * T, C).T)
    km = np.zeros((P, B), dtype=np.float32)
    for b in range(B):
        pc = (lens[b] - 1) // P            # last valid key chunk
        idx = pc * P + np.arange(P)
        km[:, b] = np.where(idx < lens[b], 0.0, NEG).astype(np.float32)

    in_maps = []
    for core in range(NCORES):
        h0 = core * HP
        in_maps.append({
            "xt": xt,
            "wq": np.ascontiguousarray(
                np.concatenate([Wq[h0 + i] for i in range(HP)], axis=1)),
            "wk": np.ascontiguousarray(
                np.concatenate([Wk[h0 + i] for i in range(HP)], axis=1)),
            "wv": np.ascontiguousarray(
                np.concatenate([Wv[h0 + i] for i in range(HP)], axis=1)),
            "wp": np.ascontiguousarray(Wp[h0 * DH:(h0 + HP) * DH, :]),
            "km": km,
        })

    res = run_bass_kernel_spmd(nc, in_maps, list(range(NCORES)))

    out = np.zeros((B * T, C), dtype=np.float32)
    for r in res.results:
        out += r["out"]
    out = out.reshape(B, T, C)
    for b in range(B):
        out[b, lens[b]:, :] = 0.0
    out += bp
    return out

